# revision 19
# baseline (speedup 1.0000x reference)
"""BiLevelRoutingAttention Trainium2 kernel (8 NeuronCores).

Sharding: core c handles (batch b = c//2, head-half hh = c%2). Each core
computes q/k/v for its 8 heads, routing (full-channel, redundant per core),
top-8 region gather, attention, LePE for its 256 v-channels, and a partial
output 1x1 conv over its 256 channels. Host sums the two partials per batch
and adds out_b.
"""
import sys
import numpy as np

sys.path.insert(0, "/opt/trn_rl_repo")

import concourse.bass as bass
import concourse.mybir as mybir
from concourse import bacc
from concourse.bass import ds
from concourse.masks import make_identity
from concourse.tile import TileContext

F32 = mybir.dt.float32
BF16 = mybir.dt.bfloat16
U32 = mybir.dt.uint32
AF = mybir.ActivationFunctionType
OP = mybir.AluOpType

B, DIM, H, W = 4, 512, 64, 64
NH = 16
NWH, NWW = 8, 8
TOPK = 8
SIDE = 5
S = H * W          # 4096 spatial
NREG = NWH * NWW   # 64 regions
RS = 64            # tokens per region
HD = 32            # head dim
NHC = 8            # heads per core
CH = NHC * HD      # 256 channels per core
SCALE = float(DIM) ** -0.5

_NC_CACHE = None


def build_nc():
    nc = bacc.Bacc(None, target_bir_lowering=False)

    x_in = nc.dram_tensor("x", [DIM, S], F32, kind="ExternalInput")
    wT_in = nc.dram_tensor("wT", [DIM, 3 * CH], F32, kind="ExternalInput")
    bqkv_in = nc.dram_tensor("bqkv", [128, 6], F32, kind="ExternalInput")
    wrT_in = nc.dram_tensor("wrT", [DIM, 2 * DIM], F32, kind="ExternalInput")
    brq_in = nc.dram_tensor("brq", [128, 4], F32, kind="ExternalInput")
    brk_in = nc.dram_tensor("brk", [128, 4], F32, kind="ExternalInput")
    wlepe_in = nc.dram_tensor("wlepe", [128, 2, 25], F32, kind="ExternalInput")
    blepe_in = nc.dram_tensor("blepe", [128, 2], F32, kind="ExternalInput")
    woT_in = nc.dram_tensor("woT", [CH, DIM], F32, kind="ExternalInput")
    y_out = nc.dram_tensor("y", [DIM, S], F32, kind="ExternalOutput")

    with TileContext(nc) as tc:
        import contextlib

        stack = contextlib.ExitStack()
        with stack:
            res = stack.enter_context(tc.tile_pool(name="res", bufs=1))
            tp = stack.enter_context(tc.tile_pool(name="tp", bufs=2))
            dram = stack.enter_context(tc.tile_pool(name="dram", bufs=1, space="DRAM"))

            # ---------------- Phase A: weights load/cast ----------------
            wT_f = tp.tile([128, 4, 3 * CH], F32, tag="wld")
            nc.sync.dma_start(wT_f[:], wT_in.ap().rearrange("(kc p) m -> p kc m", p=128))
            wT_bf = res.tile([128, 4, 3 * CH], BF16)
            nc.vector.tensor_copy(wT_bf[:], wT_f[:])

            woT_f = tp.tile([128, 2, DIM], F32, tag="wld")
            nc.sync.dma_start(woT_f[:], woT_in.ap().rearrange("(kc p) m -> p kc m", p=128))
            woT_bf = res.tile([128, 2, DIM], BF16)
            nc.vector.tensor_copy(woT_bf[:], woT_f[:])

            wrT = res.tile([128, 4, 2 * DIM], F32, tag="wrt_lepeacc")
            nc.sync.dma_start(wrT[:], wrT_in.ap().rearrange("(kc p) m -> p kc m", p=128))

            bqkv = res.tile([128, 6], F32)
            nc.sync.dma_start(bqkv[:], bqkv_in[:])
            brq = res.tile([128, 4], F32)
            nc.sync.dma_start(brq[:], brq_in[:])
            brk = res.tile([128, 4], F32)
            nc.sync.dma_start(brk[:], brk_in[:])
            wlepe = res.tile([128, 2, 25], F32)
            nc.sync.dma_start(wlepe[:], wlepe_in[:])
            blepe = res.tile([128, 2], F32)
            nc.sync.dma_start(blepe[:], blepe_in[:])

            ident = res.tile([128, 128], F32)
            make_identity(nc, ident[:])
            # block mask for recip broadcast: bm[g, 32g:32g+32] = 1
            bm = res.tile([4, 128], F32)
            nc.gpsimd.memset(bm[:], 0.0)
            # bm[g, gg*32+t] = (g == gg) ? 1 : 0
            nc.gpsimd.affine_select(
                out=bm[:].rearrange("g (gg t) -> g gg t", gg=4),
                in_=bm[:].rearrange("g (gg t) -> g gg t", gg=4),
                compare_op=OP.not_equal,
                fill=1.0,
                base=0,
                pattern=[[-1, 4], [0, 32]],
                channel_multiplier=1,
            )

            # ---------------- Phase A2: x load, cast to region-major, pool ----
            # x_bf region-major: token index = ry*512 + rx*64 + y*8 + x
            x_bf = res.tile([128, 4, S], BF16, tag="xbf_out")
            xr = res.tile([128, 4, NREG], F32)
            for c in range(4):
                xc = tp.tile([128, S], F32, tag="xload")
                nc.sync.dma_start(xc[:], x_in[c * 128 : (c + 1) * 128, :])
                for ry in range(8):
                    # out (within ry block of 512): idx = rx*64 + y*8 + x
                    o_ap = x_bf[:, c, ry * 512 : (ry + 1) * 512].rearrange(
                        "p (rx y x) -> p y rx x", rx=8, y=8, x=8
                    )
                    i_ap = xc[:, ry * 512 : (ry + 1) * 512].rearrange(
                        "p (y rx x) -> p y rx x", rx=8, y=8, x=8
                    )
                    nc.vector.tensor_copy(o_ap, i_ap)
                # pooling: sum over (y, x) per region
                r1 = tp.tile([128, 512], F32, tag="pool1")  # (ry y rx) sums over x
                for ry in range(8):
                    nc.vector.tensor_reduce(
                        r1[:, ry * 64 : (ry + 1) * 64].rearrange(
                            "p (y rx) -> p y rx", y=8
                        ),
                        xc[:, ry * 512 : (ry + 1) * 512].rearrange(
                            "p (y rx x) -> p y rx x", y=8, rx=8
                        ),
                        axis=mybir.AxisListType.X,
                        op=OP.add,
                    )
                # sum over y: in [p, ry, rx, y] (y innermost via stride tricks)
                nc.vector.tensor_reduce(
                    xr[:, c, :].rearrange("p (ry rx) -> p ry rx", ry=8),
                    r1[:].rearrange("p (ry y rx) -> p ry rx y", ry=8, y=8),
                    axis=mybir.AxisListType.X,
                    op=OP.add,
                )
            nc.vector.tensor_scalar_mul(xr[:], xr[:], 1.0 / RS)

            # ---------------- Phase B: qkv GEMM ----------------
            q_bf = res.tile([128, 2, S], BF16)
            v_ras = res.tile([128, 2, S], BF16)
            k_dram = dram.tile([2, 128, S], BF16)
            vT_dram = dram.tile([S, CH], BF16)

            gpsum_cm = tc.tile_pool(name="gpsum", bufs=4, space="PSUM")
            gpsum = gpsum_cm.__enter__()
            for m in range(6):
                for n in range(8):
                    ps = gpsum.tile([128, 512], F32, tag="g")
                    for kc in range(4):
                        nc.tensor.matmul(
                            ps[:],
                            wT_bf[:, kc, m * 128 : (m + 1) * 128],
                            x_bf[:, kc, n * 512 : (n + 1) * 512],
                            start=(kc == 0),
                            stop=(kc == 3),
                        )
                    if m < 2:  # q
                        nc.vector.tensor_scalar_add(
                            q_bf[:, m, n * 512 : (n + 1) * 512], ps[:], bqkv[:, m : m + 1]
                        )
                    elif m < 4:  # k -> DRAM ch-major
                        kt = tp.tile([128, 512], BF16, tag="kevac")
                        nc.vector.tensor_scalar_add(kt[:], ps[:], bqkv[:, m : m + 1])
                        nc.sync.dma_start(
                            k_dram[m - 2, :, n * 512 : (n + 1) * 512], kt[:]
                        )
                    else:  # v
                        vt = tp.tile([128, 512], BF16, tag="vevac")
                        nc.vector.tensor_scalar_add(vt[:], ps[:], bqkv[:, m : m + 1])
                        # raster copy for lepe: chunk n covers ry=n
                        nc.vector.tensor_copy(
                            v_ras[:, m - 4, n * 512 : (n + 1) * 512].rearrange(
                                "p (y rx x) -> p y rx x", y=8, rx=8, x=8
                            ),
                            vt[:].rearrange("p (rx y x) -> p y rx x", rx=8, y=8, x=8),
                        )
                        # transpose -> vT_dram [tok, ch]
                        vtt = tp.tile([128, 4, 128], BF16, tag="vtt")
                        for blk in range(4):
                            nc.sync.dma_start_transpose(
                                vtt[:, blk, :], vt[:, blk * 128 : (blk + 1) * 128]
                            )
                        nc.sync.dma_start(
                            vT_dram[n * 512 : (n + 1) * 512, (m - 4) * 128 : (m - 3) * 128]
                            .rearrange("(blk t) c -> t blk c", t=128),
                            vtt[:],
                        )
            gpsum_cm.__exit__(None, None, None)

            # ---------------- Phase C: routing ----------------
            with tc.tile_pool(name="rpsum", bufs=1, space="PSUM") as rpsum:
                qr = res.tile([128, 4, NREG], F32)
                kr = res.tile([128, 4, NREG], F32)
                for which, dst, bias, col0 in ((0, qr, brq, 0), (1, kr, brk, DIM)):
                    for m in range(4):
                        ps = rpsum.tile([128, NREG], F32, tag="r")
                        for kc in range(4):
                            nc.tensor.matmul(
                                ps[:],
                                wrT[:, kc, col0 + m * 128 : col0 + (m + 1) * 128],
                                xr[:, kc, :],
                                start=(kc == 0),
                                stop=(kc == 3),
                            )
                        nc.vector.tensor_scalar_add(dst[:, m, :], ps[:], bias[:, m : m + 1])
                aps = rpsum.tile([NREG, NREG], F32, tag="a")
                for kc in range(4):
                    nc.tensor.matmul(
                        aps[:], qr[:, kc, :], kr[:, kc, :], start=(kc == 0), stop=(kc == 3)
                    )
                a_sb = res.tile([NREG, NREG], F32)
                nc.vector.tensor_copy(a_sb[:], aps[:])

            amax = res.tile([NREG, 8], F32)
            aidx = res.tile([NREG, 8], U32)
            nc.vector.max(amax[:], a_sb[:])
            nc.vector.max_index(aidx[:], amax[:], a_sb[:])
            idx64 = res.tile([NREG, 8], U32)
            nc.vector.tensor_scalar_mul(idx64[:], aidx[:], RS)
            # flatten to partition 0 so value_load can read every entry
            idxf = res.tile([1, NREG * 8], U32)
            nc.sync.dma_start(
                idxf[:].rearrange("q (r s) -> q r s", r=NREG), idx64[:, None, :]
            )

            # ---------------- Phase D: attention ----------------
            out_sb = res.tile([128, 2, S], BF16, tag="xbf_out")  # raster, reuses x_bf slot

            attn_cms = [
                tc.tile_pool(name="kg", bufs=3),
                tc.tile_pool(name="vg", bufs=3),
                tc.tile_pool(name="esb", bufs=6),
                tc.tile_pool(name="etsb", bufs=6),
                tc.tile_pool(name="den", bufs=4),
                tc.tile_pool(name="rts", bufs=4),
                tc.tile_pool(name="rbs", bufs=4),
                tc.tile_pool(name="spsum", bufs=3, space="PSUM"),
                tc.tile_pool(name="avpsum", bufs=1, space="PSUM"),
                tc.tile_pool(name="rbpsum", bufs=1, space="PSUM"),
                tc.tile_pool(name="rtpsum", bufs=1, space="PSUM"),
            ]
            (kg_pool, vg_pool, e_pool, et_pool, den_pool, rt_pool, rb_pool,
             spsum, avpsum, rbpsum, rtpsum) = [cm.__enter__() for cm in attn_cms]

            avp = {}
            rbp = {}
            for rp in range(32):  # region pair
                rg4 = rp // 2
                # gathers (shared across heads)
                kg = kg_pool.tile([128, 2, 2, 512], BF16, tag="kg")
                vg = vg_pool.tile([128, 2, 4, CH], BF16, tag="vg")
                for rr in range(2):
                    r = 2 * rp + rr
                    for s_ in range(8):
                        off = nc.sync.value_load(idxf[0:1, r * 8 + s_ : r * 8 + s_ + 1])
                        nc.sync.dma_start(
                            kg[:, rr, :, s_ * 64 : (s_ + 1) * 64],
                            k_dram[:, :, ds(off, 64)].rearrange("c p t -> p c t"),
                        )
                        nc.sync.dma_start(
                            vg[(s_ % 2) * 64 : (s_ % 2) * 64 + 64, rr, s_ // 2, :],
                            vT_dram[ds(off, 64), :],
                        )
                for quad in range(2):
                    rbp_full = rbpsum.tile(
                        [128, 512], F32, tag=f"rb{quad}", name=f"rbp{quad}"
                    )
                    rbp[quad] = rbp_full[:, :128]
                for quad in range(2):
                    den = den_pool.tile([128, 4], F32, tag="den")
                    et_tiles = []
                    for hl in range(4):
                        h = quad * 4 + hl
                        sp = spsum.tile([128, 512], F32, tag="s")
                        for rr in range(2):
                            r = 2 * rp + rr
                            nc.tensor.matmul(
                                sp[rr * 64 : rr * 64 + 64, :],
                                q_bf[32 * hl : 32 * hl + 32, quad, r * 64 : (r + 1) * 64],
                                kg[32 * hl : 32 * hl + 32, rr, quad, :],
                                start=True,
                                stop=True,
                                tile_position=(32 * hl, 64 * rr),
                            )
                        esb = e_pool.tile([128, 512], BF16, tag="e")
                        nc.scalar.activation(
                            esb[:], sp[:], AF.Exp, scale=SCALE,
                            accum_out=den[:, hl : hl + 1],
                        )
                        et = et_pool.tile([128, 4, 128], BF16, tag="et")
                        for c in range(4):
                            nc.sync.dma_start_transpose(
                                et[:, c, :], esb[:, c * 128 : (c + 1) * 128]
                            )
                        et_tiles.append(et)
                    # reciprocal + transpose to [4h, 128(2r,q)]
                    rec = den_pool.tile([128, 4], F32, tag="rec")
                    nc.vector.reciprocal(rec[:], den[:])
                    rtp_full = rtpsum.tile([4, 512], F32, tag="rt", name="rtp")
                    rtp = rtp_full[:, :128]
                    nc.tensor.matmul(
                        rtp[:], rec[:], ident[:], is_transpose=True,
                        start=True, stop=True,
                    )
                    rts = rt_pool.tile([4, 128], F32, tag="rts")
                    nc.vector.tensor_copy(rts[:], rtp[:])
                    # rb broadcast matmul into psum
                    nc.tensor.matmul(
                        rbp[quad][:],
                        bm[:],
                        rts[:],
                        start=True,
                        stop=True,
                    )
                    rbs = rb_pool.tile([128, 128], F32, tag=f"rbs{quad}", name=f"rbs{quad}")
                    nc.vector.tensor_copy(rbs[:], rbp[quad][:])
                    # AV + per-region normalize/evacuate
                    for rr in range(2):
                        r = 2 * rp + rr
                        av_full = avpsum.tile(
                            [128, 512], F32, tag=f"av{quad}", name=f"avt{quad}"
                        )
                        av = av_full[:, :64]
                        for hl in range(4):
                            for c in range(4):
                                nc.tensor.matmul(
                                    av[32 * hl : 32 * hl + 32, :],
                                    vg[:, rr, c, quad * 128 + 32 * hl : quad * 128 + 32 * hl + 32],
                                    et_tiles[hl][:, c, rr * 64 : rr * 64 + 64],
                                    start=(c == 0),
                                    stop=(c == 3),
                                    tile_position=(0, 32 * hl),
                                )
                        ry, rx = r // 8, r % 8
                        o_ap = out_sb[:, quad, :].rearrange(
                            "p (ry y rx x) -> p ry y rx x", ry=8, y=8, rx=8, x=8
                        )[:, ry, :, rx, :]
                        nc.vector.scalar_tensor_tensor(
                            out=o_ap,
                            in0=av[:].rearrange("p (y x) -> p y x", y=8),
                            scalar=1.0,
                            in1=rbs[:, rr * 64 : rr * 64 + 64].rearrange(
                                "p (y x) -> p y x", y=8
                            ),
                            op0=OP.mult,
                            op1=OP.mult,
                        )

            # ---------------- Phase E: LePE ----------------
            lepe = res.tile([128, 2, S], BF16, tag="wrt_lepeacc")  # reuses wrT slot
            for c in range(2):
                # center tap (dy=0, dx=0) = tap index 12: init with bias
                nc.vector.tensor_scalar(
                    lepe[:, c, :],
                    v_ras[:, c, :],
                    wlepe[:, c, 12:13],
                    blepe[:, c : c + 1],
                    op0=OP.mult,
                    op1=OP.add,
                )
                for t in range(25):
                    if t == 12:
                        continue
                    dy, dx = t // 5 - 2, t % 5 - 2
                    y0, y1 = max(0, -dy), H - max(0, dy)
                    x0, x1 = max(0, -dx), W - max(0, dx)
                    o_ap = lepe[:, c, :].rearrange("p (y x) -> p y x", y=64)[
                        :, y0:y1, x0:x1
                    ]
                    i_ap = v_ras[:, c, :].rearrange("p (y x) -> p y x", y=64)[
                        :, y0 + dy : y1 + dy, x0 + dx : x1 + dx
                    ]
                    nc.vector.scalar_tensor_tensor(
                        out=o_ap,
                        in0=i_ap,
                        scalar=wlepe[:, c, t : t + 1],
                        in1=o_ap,
                        op0=OP.mult,
                        op1=OP.add,
                    )
            # add lepe into out_sb (per n-chunk for pipelining)
            for n in range(8):
                nc.vector.tensor_tensor(
                    out_sb[:, :, n * 512 : (n + 1) * 512],
                    out_sb[:, :, n * 512 : (n + 1) * 512],
                    lepe[:, :, n * 512 : (n + 1) * 512],
                    OP.add,
                )

            # ---------------- Phase F: output conv ----------------
            for cm in reversed(attn_cms):
                cm.__exit__(None, None, None)
            opsum = stack.enter_context(tc.tile_pool(name="opsum", bufs=4, space="PSUM"))
            for m in range(4):
                for n in range(8):
                    ps = opsum.tile([128, 512], F32, tag="o")
                    for kc in range(2):
                        nc.tensor.matmul(
                            ps[:],
                            woT_bf[:, kc, m * 128 : (m + 1) * 128],
                            out_sb[:, kc, n * 512 : (n + 1) * 512],
                            start=(kc == 0),
                            stop=(kc == 1),
                        )
                    yt = tp.tile([128, 512], F32, tag="yevac")
                    nc.vector.tensor_copy(yt[:], ps[:])
                    nc.sync.dma_start(
                        y_out[m * 128 : (m + 1) * 128, n * 512 : (n + 1) * 512], yt[:]
                    )

    nc.compile()
    return nc


def _get_nc():
    global _NC_CACHE
    if _NC_CACHE is None:
        _NC_CACHE = build_nc()
    return _NC_CACHE


def _prep_core_inputs(inputs, b, hh):
    x = np.asarray(inputs["x"])
    qkv_w = np.asarray(inputs["qkv_w"])
    qkv_b = np.asarray(inputs["qkv_b"])
    lepe_w = np.asarray(inputs["lepe_w"])
    lepe_b = np.asarray(inputs["lepe_b"])
    out_w = np.asarray(inputs["out_w"])

    rows = np.concatenate(
        [
            np.arange(hh * CH, (hh + 1) * CH),
            np.arange(DIM + hh * CH, DIM + (hh + 1) * CH),
            np.arange(2 * DIM + hh * CH, 2 * DIM + (hh + 1) * CH),
        ]
    )
    wT = np.ascontiguousarray(qkv_w[rows].T)                     # [512, 768]
    bqkv = np.ascontiguousarray(qkv_b[rows].reshape(6, 128).T)   # [128, 6]
    wrT = np.ascontiguousarray(qkv_w[: 2 * DIM].T)               # [512, 1024]
    brq = np.ascontiguousarray(qkv_b[:DIM].reshape(4, 128).T)    # [128, 4]
    brk = np.ascontiguousarray(qkv_b[DIM : 2 * DIM].reshape(4, 128).T)
    wl = lepe_w.reshape(DIM, 25)[hh * CH : (hh + 1) * CH]
    wlepe = np.ascontiguousarray(wl.reshape(2, 128, 25).transpose(1, 0, 2))  # [128, 2, 25]
    blepe = np.ascontiguousarray(
        lepe_b[hh * CH : (hh + 1) * CH].reshape(2, 128).T
    )  # [128, 2]
    woT = np.ascontiguousarray(out_w[:, hh * CH : (hh + 1) * CH].T)  # [256, 512]

    return {
        "x": np.ascontiguousarray(x[b].reshape(DIM, S), dtype=np.float32),
        "wT": wT.astype(np.float32),
        "bqkv": bqkv.astype(np.float32),
        "wrT": wrT.astype(np.float32),
        "brq": brq.astype(np.float32),
        "brk": brk.astype(np.float32),
        "wlepe": wlepe.astype(np.float32),
        "blepe": blepe.astype(np.float32),
        "woT": woT.astype(np.float32),
    }


def kernel(**inputs) -> np.ndarray:
    from concourse.bass_utils import run_bass_kernel_spmd

    nc = _get_nc()
    in_maps = [_prep_core_inputs(inputs, c // 2, c % 2) for c in range(8)]
    res = run_bass_kernel_spmd(nc, in_maps, core_ids=list(range(8)))
    out_b = np.asarray(inputs["out_b"])
    y = np.zeros((B, DIM, H, W), dtype=np.float32)
    for b in range(B):
        part = res.results[2 * b]["y"] + res.results[2 * b + 1]["y"]
        y[b] = part.reshape(DIM, H, W) + out_b[:, None, None]
    return y
